# revision 24
# baseline (speedup 1.0000x reference)
"""Trainium2 Bass kernel for nn_AggregateClusteredSum (segment_reduce).

Computes, per batch row b:
    cs  = cs_o[0] with positions >= n set to -1
    Hk[k]  = sum_{i: cs[i]==k} hs[b, i, :]                  (K segment sums)
    H      = [Hk ; Hk + hn ; hn]   with hn = hs[b, n, :]    ([2K+1, H])
    gs     = relu(H @ W1 + b1) @ W2 + b2                    ([2K+1, GD])
    S      = sum_k gs[:K]
    G      = [S - gs[:K] + gs[K:2K] ; S + gs[2K]]           ([K+1, GD])
returns (G, ones(B, K+1)).

Strategy: data-parallel over batch across 8 NeuronCores (B=32 -> 4 rows per
core).  The dominant cost is streaming hs (32 MiB/core) from HBM once; the
segment reduction is done on the TensorEngine as an accumulated onehot matmul
while chunks stream in.  To keep the PE off the slow fp32 path (4 cycles/row)
hs is split on the fly into bf16 hi + bf16 lo (exact to ~2^-17 relative) and
two bf16 matmuls accumulate into the same fp32 PSUM tile.  The small MLP +
combine run per batch on the transposed [H, 2K+1] layout so all bias/hn
broadcasts are per-partition scalars.
"""

import os
import sys

import numpy as np

for _p in ("/opt/trn_rl_repo", os.path.expanduser("~/.axon_site/_ro/trn_rl_repo")):
    if os.path.isdir(_p) and _p not in sys.path:
        sys.path.insert(0, _p)

import concourse.bass as bass
import concourse.tile as tile
from concourse import mybir
from concourse.bass import ts
from concourse.masks import make_identity

N_CORES = 8
PART = 128
M_PER = 16  # hs rows held per partition per chunk (chunk = 2 MiB)

# Segment-matmul operand mode: 'bf16hilo' (exact-ish split, fast PE),
# 'f32' (exact, 4x PE cost), 'f32r' (single-pass fp32, HW-reduced precision),
# 'bf16' (single bf16 pass, on-chip cast), 'bf16dma' (single bf16 pass,
# cast during the HBM->SBUF DMA).
SEG_MODE = os.environ.get("AGG_SEG_MODE", "bf16hilo")
TRACE = bool(int(os.environ.get("AGG_TRACE", "0")))

LAST_EXEC_NS = None
LAST_RESULTS = None


def _split_sync_waits(nc, max_waits=1):
    """Walrus codegen in this toolchain rejects instructions carrying more
    than one semaphore wait ("Too many sync wait commands" in CoreV3 crash,
    seen on Tile's kernel-tail Drain).  Sem waits are AND conditions, so an
    instruction with N waits is equivalent to N-1 same-engine nop+wait
    carriers followed by the instruction with the last wait."""
    from concourse import mybir as _mb

    n_split = 0
    for fn in nc.m.functions:
        for bb in fn.blocks:
            insts = bb.instructions
            out = []
            changed = False
            for inst in insts:
                si = inst.sync_info
                if si is not None and si.on_wait and len(si.on_wait) > max_waits:
                    waits = list(si.on_wait)
                    head, tail = waits[:-max_waits], waits[-max_waits:]
                    for i in range(0, len(head), max_waits):
                        out.append(
                            _mb.InstNoOp(
                                name=f"{inst.name}-sw{i}",
                                engine=inst.engine,
                                bass_nofuse=True,
                                sync_info=_mb.SyncInfo(
                                    on_wait=head[i : i + max_waits], on_update=[]
                                ),
                                ins=[],
                                outs=[],
                            )
                        )
                    inst.sync_info = _mb.SyncInfo(
                        on_wait=tail, on_update=list(si.on_update)
                    )
                    changed = True
                    n_split += 1
                out.append(inst)
            if changed:
                bb.instructions = out
    return n_split


def _build(B_LOC, N, H, GH, GD, K, n, seg_mode, split_waits=True):
    f32 = mybir.dt.float32
    bf16 = mybir.dt.bfloat16
    i32 = mybir.dt.int32
    PSUM = bass.MemorySpace.PSUM

    assert N % (PART * M_PER) == 0
    NCHUNK = N // (PART * M_PER)
    assert H % PART == 0 and GH % PART == 0 and GD % PART == 0
    JH, JG, JD = H // PART, GH // PART, GD // PART
    assert K <= PART and 2 * K + 1 <= 512
    K1, K2 = K + 1, 2 * K + 1
    assert 0 <= n < N

    w_dt = {"f32": f32, "f32r": f32, "bf16": bf16, "bf16dma": bf16, "bf16hilo": bf16}[
        seg_mode
    ]

    nc = bass.Bass()
    hs = nc.dram_tensor("hs", [B_LOC, N, H], f32, kind="ExternalInput")
    cs = nc.dram_tensor("cs", [N], i32, kind="ExternalInput")
    W1 = nc.dram_tensor("W1", [H, GH], f32, kind="ExternalInput")
    b1 = nc.dram_tensor("b1", [GH], f32, kind="ExternalInput")
    W2 = nc.dram_tensor("W2", [GH, GD], f32, kind="ExternalInput")
    b2 = nc.dram_tensor("b2", [GD], f32, kind="ExternalInput")
    out = nc.dram_tensor("out", [B_LOC, K1, GD], f32, kind="ExternalOutput")

    with tile.TileContext(nc) as tc:
        with (
            tc.tile_pool(name="const", bufs=1) as const,
            tc.tile_pool(name="chunks", bufs=4) as chunks,
            tc.tile_pool(name="conv", bufs=3) as conv,
            tc.tile_pool(name="epi", bufs=2) as epi,
            tc.tile_pool(name="ps_hk", bufs=2, space=PSUM) as ps_hk,
            tc.tile_pool(name="ps_t", bufs=2, space=PSUM) as ps_t,
            tc.tile_pool(name="ps_mlp", bufs=2, space=PSUM) as ps_mlp,
        ):
            # ---------------- constants ----------------
            ident = const.tile([PART, PART], f32)
            make_identity(nc, ident)

            # iota over k (exact for k < 2^24 in f32)
            iota_k = const.tile([PART, K], f32)
            nc.gpsimd.iota(
                iota_k,
                pattern=[[1, K]],
                base=0,
                channel_multiplier=0,
                allow_small_or_imprecise_dtypes=True,
            )

            # row index of (p, c, m) in the chunked hs layout
            rowidx = const.tile([PART, NCHUNK, M_PER], i32)
            nc.gpsimd.iota(
                rowidx,
                pattern=[[PART * M_PER, NCHUNK], [1, M_PER]],
                base=0,
                channel_multiplier=M_PER,
            )

            # cluster labels in the same (p, c, m) layout
            cs_i = const.tile([PART, NCHUNK, M_PER], i32)
            nc.scalar.dma_start(
                out=cs_i, in_=cs.rearrange("(c p m) -> p c m", p=PART, m=M_PER)
            )
            cs_f = const.tile([PART, NCHUNK, M_PER], f32)
            nc.gpsimd.tensor_copy(cs_f, cs_i)

            # rows >= n get label -1 (excluded from every cluster)
            gemask = const.tile([PART, NCHUNK, M_PER], i32)
            nc.gpsimd.tensor_scalar(
                out=gemask,
                in0=rowidx,
                scalar1=n,
                scalar2=None,
                op0=mybir.AluOpType.is_ge,
            )
            negones = const.tile([PART, NCHUNK, M_PER], f32)
            nc.vector.memset(negones, -1.0)
            nc.vector.copy_predicated(cs_f, gemask, negones)

            # onehot matmul weights: w[p, (c, m), k] = (cs[row(p,c,m)] == k).
            # One fused broadcast-AP compare per chunk (DVE) instead of
            # NCHUNK*M_PER small tensor_scalar ops.
            w_sb = const.tile([PART, NCHUNK, M_PER, K], w_dt)
            for c in range(NCHUNK):
                cs_b = cs_f[:, c, :].unsqueeze(2).broadcast_to([PART, M_PER, K])
                iota_b = iota_k.unsqueeze(1).broadcast_to([PART, M_PER, K])
                nc.vector.tensor_tensor(
                    out=w_sb[:, c], in0=cs_b, in1=iota_b, op=mybir.AluOpType.is_equal
                )

            # MLP weights, partition-major on the contraction dim
            w1_sb = const.tile([PART, JH, GH], f32)
            nc.scalar.dma_start(out=w1_sb, in_=W1.rearrange("(i p) j -> p i j", p=PART))
            w2_sb = const.tile([PART, JG, GD], f32)
            nc.scalar.dma_start(out=w2_sb, in_=W2.rearrange("(i p) j -> p i j", p=PART))
            b1t = const.tile([PART, JG], f32)
            nc.scalar.dma_start(out=b1t, in_=b1.rearrange("(j p) -> p j", p=PART))
            b2t = const.tile([PART, JD], f32)
            nc.scalar.dma_start(out=b2t, in_=b2.rearrange("(j p) -> p j", p=PART))

            hs_r = hs.rearrange("b (c p m) h -> b c p m h", p=PART, m=M_PER)
            n_seg_mm = NCHUNK * M_PER * (2 if seg_mode == "bf16hilo" else 1)

            def load_chunk(b, c):
                """Issue the chunk DMA + hi/lo conversion; return the rhs
                tile(s) for the segment matmuls.  Split out so batch 0's DMAs
                can be traced before the const loads (SP HWDGE is FIFO in
                trace order)."""
                if seg_mode == "bf16dma":
                    hi = conv.tile([PART, M_PER, H], bf16, tag="hi")
                    nc.gpsimd.dma_start(out=hi, in_=hs_r[b, c])
                    return hi
                ch = chunks.tile([PART, M_PER, H], f32, tag="ch")
                nc.sync.dma_start(out=ch, in_=hs_r[b, c])
                ch_flat = ch.rearrange("p m h -> p (m h)")
                if seg_mode == "bf16hilo":
                    # hi and lo packed in one tile so each (c, m) needs ONE
                    # matmul with free dim 2*H: out[k, {hi,lo}, h] — the two
                    # halves are summed in the epilogue.  hi is built on the
                    # (otherwise idle) scalar engine, lo on DVE, both through
                    # flat 2D APs (3D APs pay per-subblock overhead).
                    hl = conv.tile([PART, 2, M_PER, H], bf16, tag="hl")
                    hi_flat = hl[:, 0].rearrange("p m h -> p (m h)")
                    nc.scalar.copy(hi_flat, ch_flat)
                    nc.vector.tensor_sub(
                        hl[:, 1].rearrange("p m h -> p (m h)"), ch_flat, hi_flat
                    )
                    return hl
                if seg_mode == "bf16":
                    hi = conv.tile([PART, M_PER, H], bf16, tag="hi")
                    nc.scalar.copy(hi.rearrange("p m h -> p (m h)"), ch_flat)
                    return hi
                return ch

            def stream_batch(b, preloaded=None):
                # ---- segment sums: Hk = onehot @ hs[b], PSUM-accumulated ----
                hilo = seg_mode == "bf16hilo"
                psum_hk = ps_hk.tile([K, 2, H] if hilo else [K, H], f32, tag="hk")
                n_mm = NCHUNK * M_PER
                mm = 0
                for c in range(NCHUNK):
                    r = preloaded[c] if preloaded else load_chunk(b, c)
                    for m in range(M_PER):
                        lhsT = w_sb[:, c, m, :]
                        rhs = r[:, :, m, :] if hilo else r[:, m, :]
                        if seg_mode == "f32r":
                            lhsT = lhsT.bitcast(mybir.dt.float32r)
                            rhs = rhs.bitcast(mybir.dt.float32r)
                        nc.tensor.matmul(
                            psum_hk,
                            lhsT,
                            rhs,
                            start=(mm == 0),
                            stop=(mm == n_mm - 1),
                        )
                        mm += 1
                return psum_hk

            def epilogue(b, psum_hk):
                # ---- epilogue on transposed [H, *] layout ----
                hk_sb = epi.tile([K, H], f32, tag="hk_sb")
                if seg_mode == "bf16hilo":
                    # sum the packed hi|lo halves while moving PSUM -> SBUF
                    # (only one PSUM operand allowed per DVE op)
                    nc.scalar.copy(hk_sb, psum_hk[:, 0, :])
                    nc.vector.tensor_add(hk_sb, hk_sb, psum_hk[:, 1, :])
                else:
                    nc.scalar.copy(hk_sb, psum_hk)

                hnt = epi.tile([PART, JH], f32, tag="hnt")
                nc.scalar.dma_start(
                    out=hnt, in_=hs[b, n].rearrange("(j p) -> p j", p=PART)
                )

                # HkT via PE transpose: [K, H] -> JH x [PART, K]
                ps_tr = ps_t.tile([PART, JH, K], f32, tag="tr")
                for j in range(JH):
                    nc.tensor.transpose(
                        ps_tr[:, j, :], hk_sb[:, ts(j, PART)], ident[0:K, 0:K]
                    )

                # H^T columns: [0:K]=Hk, [K:2K]=Hk+hn, [2K]=hn
                ht = epi.tile([PART, JH, K2], f32, tag="ht")
                for j in range(JH):
                    nc.vector.tensor_copy(ht[:, j, 0:K], ps_tr[:, j, :])
                    nc.vector.tensor_scalar_add(
                        ht[:, j, K : 2 * K], ps_tr[:, j, :], hnt[:, j : j + 1]
                    )
                    nc.vector.tensor_copy(ht[:, j, 2 * K : K2], hnt[:, j : j + 1])

                # layer 1: z1^T = W1^T @ H^T ; g1 = relu(z1 + b1)
                g1 = epi.tile([PART, JG, K2], f32, tag="g1")
                for jj in range(JG):
                    pz1 = ps_mlp.tile([PART, K2], f32, tag="mlp")
                    for i in range(JH):
                        nc.tensor.matmul(
                            pz1,
                            w1_sb[:, i, ts(jj, PART)],
                            ht[:, i, :],
                            start=(i == 0),
                            stop=(i == JH - 1),
                        )
                    nc.scalar.activation(
                        g1[:, jj, :],
                        pz1,
                        func=mybir.ActivationFunctionType.Relu,
                        bias=b1t[:, jj : jj + 1],
                        scale=1.0,
                    )

                # layer 2: gs^T = W2^T @ g1 + b2
                gs = epi.tile([PART, JD, K2], f32, tag="gs")
                for j in range(JD):
                    pz2 = ps_mlp.tile([PART, K2], f32, tag="mlp")
                    for i in range(JG):
                        nc.tensor.matmul(
                            pz2,
                            w2_sb[:, i, ts(j, PART)],
                            g1[:, i, :],
                            start=(i == 0),
                            stop=(i == JG - 1),
                        )
                    nc.vector.tensor_scalar_add(gs[:, j, :], pz2, b2t[:, j : j + 1])

                # combine: G^T[:, :K] = S - gs[:K] + gs[K:2K]; G^T[:, K] = S + gs[2K]
                gt = epi.tile([PART, JD, K1], f32, tag="gt")
                ssum = epi.tile([PART, JD], f32, tag="ssum")
                for j in range(JD):
                    nc.vector.reduce_sum(
                        ssum[:, j : j + 1], gs[:, j, 0:K], axis=mybir.AxisListType.X
                    )
                    nc.vector.tensor_sub(gt[:, j, 0:K], gs[:, j, K : 2 * K], gs[:, j, 0:K])
                    nc.vector.tensor_scalar_add(
                        gt[:, j, 0:K], gt[:, j, 0:K], ssum[:, j : j + 1]
                    )
                    nc.vector.tensor_add(
                        gt[:, j, K:K1], gs[:, j, 2 * K : 2 * K + 1], ssum[:, j : j + 1]
                    )

                # back to row-major [K+1, GD] and store
                gout = epi.tile([K1, JD, PART], f32, tag="gout")
                for j in range(JD):
                    pg = ps_mlp.tile([K1, PART], f32, tag="mlp")
                    nc.tensor.transpose(pg, gt[:, j, :], ident)
                    nc.scalar.copy(gout[:, j, :], pg)
                nc.scalar.dma_start(
                    out=out[b], in_=gout.rearrange("k j h -> k (j h)")
                )

            # Defer each batch's epilogue until after the NEXT batch's
            # streaming matmuls are traced: the PE executes in program order,
            # so placing the epilogue's matmuls behind batch b+1's stream
            # lets its cross-engine chain (ACT copy -> PE transpose -> DVE ->
            # PE MLP) fill while the PE keeps grinding stream matmuls,
            # instead of stalling the PE (and starving the DMA pipeline) at
            # every batch boundary.
            prev = None
            for b in range(B_LOC):
                hk = stream_batch(b)
                if prev is not None:
                    epilogue(b - 1, prev)
                prev = hk
            epilogue(B_LOC - 1, prev)

    nc.finalize()
    if split_waits:
        # needed by this toolchain's walrus; CoreSim can't simulate the
        # injected carriers, so sim callers pass split_waits=False
        _split_sync_waits(nc)
    return nc


def kernel(hs, cs_o, W1, b1, W2, b2, n, K):
    global LAST_EXEC_NS, LAST_RESULTS
    from concourse.bass_utils import run_bass_kernel_spmd

    hs = np.ascontiguousarray(np.asarray(hs, dtype=np.float32))
    cs_row = np.ascontiguousarray(np.asarray(cs_o)[0].astype(np.int32))
    W1 = np.ascontiguousarray(np.asarray(W1, dtype=np.float32))
    b1 = np.ascontiguousarray(np.asarray(b1, dtype=np.float32))
    W2 = np.ascontiguousarray(np.asarray(W2, dtype=np.float32))
    b2 = np.ascontiguousarray(np.asarray(b2, dtype=np.float32))
    n = int(n)
    K = int(K)

    B, N, H = hs.shape
    GH = W1.shape[1]
    GD = W2.shape[1]
    assert B % N_CORES == 0
    B_LOC = B // N_CORES

    nc = _build(B_LOC, N, H, GH, GD, K, n, SEG_MODE)

    in_maps = []
    for i in range(N_CORES):
        in_maps.append(
            {
                "hs": hs[i * B_LOC : (i + 1) * B_LOC],
                "cs": cs_row,
                "W1": W1,
                "b1": b1,
                "W2": W2,
                "b2": b2,
            }
        )

    tmpdir = os.environ.get("AGG_TMPDIR") or None
    res = run_bass_kernel_spmd(
        nc, in_maps, core_ids=list(range(N_CORES)), trace=TRACE, tmpdir=tmpdir
    )
    LAST_EXEC_NS = res.exec_time_ns
    LAST_RESULTS = res
    G = np.concatenate([res.results[i]["out"] for i in range(N_CORES)], axis=0)
    G_mask = np.ones((B, K + 1), dtype=np.float32)
    return G, G_mask


# revision 26
# speedup vs baseline: 1.1567x; 1.1567x over previous
"""Trainium2 Bass kernel for nn_AggregateClusteredSum (segment_reduce).

Computes, per batch row b:
    cs  = cs_o[0] with positions >= n set to -1
    Hk[k]  = sum_{i: cs[i]==k} hs[b, i, :]                  (K segment sums)
    H      = [Hk ; Hk + hn ; hn]   with hn = hs[b, n, :]    ([2K+1, H])
    gs     = relu(H @ W1 + b1) @ W2 + b2                    ([2K+1, GD])
    S      = sum_k gs[:K]
    G      = [S - gs[:K] + gs[K:2K] ; S + gs[2K]]           ([K+1, GD])
returns (G, ones(B, K+1)).

Strategy: data-parallel over batch across 8 NeuronCores (B=32 -> 4 rows per
core).  The dominant cost is streaming hs (32 MiB/core) from HBM once; the
segment reduction is done on the TensorEngine as an accumulated onehot matmul
while chunks stream in.  To keep the PE off the slow fp32 path (4 cycles/row)
hs is split on the fly into bf16 hi + bf16 lo (exact to ~2^-17 relative) and
two bf16 matmuls accumulate into the same fp32 PSUM tile.  The small MLP +
combine run per batch on the transposed [H, 2K+1] layout so all bias/hn
broadcasts are per-partition scalars.
"""

import os
import sys

import numpy as np

for _p in ("/opt/trn_rl_repo", os.path.expanduser("~/.axon_site/_ro/trn_rl_repo")):
    if os.path.isdir(_p) and _p not in sys.path:
        sys.path.insert(0, _p)

import concourse.bass as bass
import concourse.tile as tile
from concourse import mybir
from concourse.bass import ts
from concourse.masks import make_identity

N_CORES = 8
PART = 128
M_PER = 16  # hs rows held per partition per chunk (chunk = 2 MiB)

# Segment-matmul operand mode: 'bf16hilo' (exact-ish split, fast PE),
# 'f32' (exact, 4x PE cost), 'f32r' (single-pass fp32, HW-reduced precision),
# 'bf16' (single bf16 pass, on-chip cast), 'bf16dma' (single bf16 pass,
# cast during the HBM->SBUF DMA).
SEG_MODE = os.environ.get("AGG_SEG_MODE", "bf16hilo")
TRACE = bool(int(os.environ.get("AGG_TRACE", "0")))

LAST_EXEC_NS = None
LAST_RESULTS = None


def _split_sync_waits(nc, max_waits=1):
    """Walrus codegen in this toolchain rejects instructions carrying more
    than one semaphore wait ("Too many sync wait commands" in CoreV3 crash,
    seen on Tile's kernel-tail Drain).  Sem waits are AND conditions, so an
    instruction with N waits is equivalent to N-1 same-engine nop+wait
    carriers followed by the instruction with the last wait."""
    from concourse import mybir as _mb

    n_split = 0
    for fn in nc.m.functions:
        for bb in fn.blocks:
            insts = bb.instructions
            out = []
            changed = False
            for inst in insts:
                si = inst.sync_info
                if si is not None and si.on_wait and len(si.on_wait) > max_waits:
                    waits = list(si.on_wait)
                    head, tail = waits[:-max_waits], waits[-max_waits:]
                    for i in range(0, len(head), max_waits):
                        out.append(
                            _mb.InstNoOp(
                                name=f"{inst.name}-sw{i}",
                                engine=inst.engine,
                                bass_nofuse=True,
                                sync_info=_mb.SyncInfo(
                                    on_wait=head[i : i + max_waits], on_update=[]
                                ),
                                ins=[],
                                outs=[],
                            )
                        )
                    inst.sync_info = _mb.SyncInfo(
                        on_wait=tail, on_update=list(si.on_update)
                    )
                    changed = True
                    n_split += 1
                out.append(inst)
            if changed:
                bb.instructions = out
    return n_split


def _build(B_LOC, N, H, GH, GD, K, n, seg_mode, split_waits=True):
    f32 = mybir.dt.float32
    bf16 = mybir.dt.bfloat16
    i32 = mybir.dt.int32
    PSUM = bass.MemorySpace.PSUM

    assert N % (PART * M_PER) == 0
    NCHUNK = N // (PART * M_PER)
    assert H % PART == 0 and GH % PART == 0 and GD % PART == 0
    JH, JG, JD = H // PART, GH // PART, GD // PART
    assert K <= PART and 2 * K + 1 <= 512
    K1, K2 = K + 1, 2 * K + 1
    assert 0 <= n < N

    w_dt = {"f32": f32, "f32r": f32, "bf16": bf16, "bf16dma": bf16, "bf16hilo": bf16}[
        seg_mode
    ]

    nc = bass.Bass()
    hs = nc.dram_tensor("hs", [B_LOC, N, H], f32, kind="ExternalInput")
    cs = nc.dram_tensor("cs", [N], i32, kind="ExternalInput")
    W1 = nc.dram_tensor("W1", [H, GH], f32, kind="ExternalInput")
    b1 = nc.dram_tensor("b1", [GH], f32, kind="ExternalInput")
    W2 = nc.dram_tensor("W2", [GH, GD], f32, kind="ExternalInput")
    b2 = nc.dram_tensor("b2", [GD], f32, kind="ExternalInput")
    out = nc.dram_tensor("out", [B_LOC, K1, GD], f32, kind="ExternalOutput")

    with tile.TileContext(nc) as tc:
        with (
            tc.tile_pool(name="const", bufs=1) as const,
            tc.tile_pool(name="chunks", bufs=4) as chunks,
            tc.tile_pool(name="conv", bufs=4) as conv,
            tc.tile_pool(name="epi", bufs=2) as epi,
            tc.tile_pool(name="ps_hk", bufs=2, space=PSUM) as ps_hk,
            tc.tile_pool(name="ps_t", bufs=2, space=PSUM) as ps_t,
            tc.tile_pool(name="ps_mlp", bufs=2, space=PSUM) as ps_mlp,
        ):
            # ---------------- constants ----------------
            ident = const.tile([PART, PART], f32)
            make_identity(nc, ident)

            # iota over k (exact for k < 2^24 in f32)
            iota_k = const.tile([PART, K], f32)
            nc.gpsimd.iota(
                iota_k,
                pattern=[[1, K]],
                base=0,
                channel_multiplier=0,
                allow_small_or_imprecise_dtypes=True,
            )

            # row index of (p, c, m) in the chunked hs layout
            rowidx = const.tile([PART, NCHUNK, M_PER], i32)
            nc.gpsimd.iota(
                rowidx,
                pattern=[[PART * M_PER, NCHUNK], [1, M_PER]],
                base=0,
                channel_multiplier=M_PER,
            )

            # cluster labels in the same (p, c, m) layout
            cs_i = const.tile([PART, NCHUNK, M_PER], i32)
            nc.scalar.dma_start(
                out=cs_i, in_=cs.rearrange("(c p m) -> p c m", p=PART, m=M_PER)
            )
            cs_f = const.tile([PART, NCHUNK, M_PER], f32)
            nc.gpsimd.tensor_copy(cs_f, cs_i)

            # rows >= n get label -1 (excluded from every cluster)
            gemask = const.tile([PART, NCHUNK, M_PER], i32)
            nc.gpsimd.tensor_scalar(
                out=gemask,
                in0=rowidx,
                scalar1=n,
                scalar2=None,
                op0=mybir.AluOpType.is_ge,
            )
            negones = const.tile([PART, NCHUNK, M_PER], f32)
            nc.vector.memset(negones, -1.0)
            nc.vector.copy_predicated(cs_f, gemask, negones)

            # onehot matmul weights: w[p, (c, m), k] = (cs[row(p,c,m)] == k).
            # One fused broadcast-AP compare per chunk (DVE) instead of
            # NCHUNK*M_PER small tensor_scalar ops.
            w_sb = const.tile([PART, NCHUNK, M_PER, K], w_dt)
            for c in range(NCHUNK):
                cs_b = cs_f[:, c, :].unsqueeze(2).broadcast_to([PART, M_PER, K])
                iota_b = iota_k.unsqueeze(1).broadcast_to([PART, M_PER, K])
                nc.vector.tensor_tensor(
                    out=w_sb[:, c], in0=cs_b, in1=iota_b, op=mybir.AluOpType.is_equal
                )

            # MLP weights, partition-major on the contraction dim
            w1_sb = const.tile([PART, JH, GH], f32)
            nc.scalar.dma_start(out=w1_sb, in_=W1.rearrange("(i p) j -> p i j", p=PART))
            w2_sb = const.tile([PART, JG, GD], f32)
            nc.scalar.dma_start(out=w2_sb, in_=W2.rearrange("(i p) j -> p i j", p=PART))
            b1t = const.tile([PART, JG], f32)
            nc.scalar.dma_start(out=b1t, in_=b1.rearrange("(j p) -> p j", p=PART))
            b2t = const.tile([PART, JD], f32)
            nc.scalar.dma_start(out=b2t, in_=b2.rearrange("(j p) -> p j", p=PART))

            hs_r = hs.rearrange("b (c p m) h -> b c p m h", p=PART, m=M_PER)
            n_seg_mm = NCHUNK * M_PER * (2 if seg_mode == "bf16hilo" else 1)

            def load_chunk(b, c):
                """Issue the chunk DMA + hi/lo conversion; return the rhs
                tile(s) for the segment matmuls.  Split out so batch 0's DMAs
                can be traced before the const loads (SP HWDGE is FIFO in
                trace order)."""
                if seg_mode == "bf16dma":
                    hi = conv.tile([PART, M_PER, H], bf16, tag="hi")
                    nc.gpsimd.dma_start(out=hi, in_=hs_r[b, c])
                    return hi
                ch = chunks.tile([PART, M_PER, H], f32, tag="ch")
                nc.sync.dma_start(out=ch, in_=hs_r[b, c])
                ch_flat = ch.rearrange("p m h -> p (m h)")
                if seg_mode == "bf16hilo":
                    # hi and lo packed in one tile so each (c, m) needs ONE
                    # matmul with free dim 2*H: out[k, {hi,lo}, h] — the two
                    # halves are summed in the epilogue.  hi is built on the
                    # (otherwise idle) scalar engine, lo on DVE, both through
                    # flat 2D APs (3D APs pay per-subblock overhead).
                    hl = conv.tile([PART, 2, M_PER, H], bf16, tag="hl")
                    hi_flat = hl[:, 0].rearrange("p m h -> p (m h)")
                    lo_flat = hl[:, 1].rearrange("p m h -> p (m h)")
                    # two half-chunk ops so the DVE subtract of half 0
                    # overlaps the ACT copy of half 1 (shorter critical path
                    # at the kernel tail)
                    half = M_PER * H // 2
                    for q in range(2):
                        sl = ts(q, half)
                        nc.scalar.copy(hi_flat[:, sl], ch_flat[:, sl])
                        nc.vector.tensor_sub(
                            lo_flat[:, sl], ch_flat[:, sl], hi_flat[:, sl]
                        )
                    return hl
                if seg_mode == "bf16":
                    hi = conv.tile([PART, M_PER, H], bf16, tag="hi")
                    nc.scalar.copy(hi.rearrange("p m h -> p (m h)"), ch_flat)
                    return hi
                return ch

            def stream_batch(b, preloaded=None):
                # ---- segment sums: Hk = onehot @ hs[b], PSUM-accumulated ----
                hilo = seg_mode == "bf16hilo"
                psum_hk = ps_hk.tile([K, 2, H] if hilo else [K, H], f32, tag="hk")
                n_mm = NCHUNK * M_PER
                mm = 0
                for c in range(NCHUNK):
                    r = preloaded[c] if preloaded else load_chunk(b, c)
                    for m in range(M_PER):
                        lhsT = w_sb[:, c, m, :]
                        rhs = r[:, :, m, :] if hilo else r[:, m, :]
                        if seg_mode == "f32r":
                            lhsT = lhsT.bitcast(mybir.dt.float32r)
                            rhs = rhs.bitcast(mybir.dt.float32r)
                        nc.tensor.matmul(
                            psum_hk,
                            lhsT,
                            rhs,
                            start=(mm == 0),
                            stop=(mm == n_mm - 1),
                        )
                        mm += 1
                return psum_hk

            def epilogue(b, psum_hk):
                # ---- epilogue on transposed [H, *] layout ----
                hk_sb = epi.tile([K, H], f32, tag="hk_sb")
                if seg_mode == "bf16hilo":
                    # sum the packed hi|lo halves while moving PSUM -> SBUF
                    # (only one PSUM operand allowed per DVE op)
                    nc.scalar.copy(hk_sb, psum_hk[:, 0, :])
                    nc.vector.tensor_add(hk_sb, hk_sb, psum_hk[:, 1, :])
                else:
                    nc.scalar.copy(hk_sb, psum_hk)

                hnt = epi.tile([PART, JH], f32, tag="hnt")
                nc.scalar.dma_start(
                    out=hnt, in_=hs[b, n].rearrange("(j p) -> p j", p=PART)
                )

                # HkT via PE transpose: [K, H] -> JH x [PART, K]
                ps_tr = ps_t.tile([PART, JH, K], f32, tag="tr")
                for j in range(JH):
                    nc.tensor.transpose(
                        ps_tr[:, j, :], hk_sb[:, ts(j, PART)], ident[0:K, 0:K]
                    )

                # H^T columns: [0:K]=Hk, [K:2K]=Hk+hn, [2K]=hn
                ht = epi.tile([PART, JH, K2], f32, tag="ht")
                for j in range(JH):
                    nc.vector.tensor_copy(ht[:, j, 0:K], ps_tr[:, j, :])
                    nc.vector.tensor_scalar_add(
                        ht[:, j, K : 2 * K], ps_tr[:, j, :], hnt[:, j : j + 1]
                    )
                    nc.vector.tensor_copy(ht[:, j, 2 * K : K2], hnt[:, j : j + 1])

                # layer 1: z1^T = W1^T @ H^T ; g1 = relu(z1 + b1)
                g1 = epi.tile([PART, JG, K2], f32, tag="g1")
                for jj in range(JG):
                    pz1 = ps_mlp.tile([PART, K2], f32, tag="mlp")
                    for i in range(JH):
                        nc.tensor.matmul(
                            pz1,
                            w1_sb[:, i, ts(jj, PART)],
                            ht[:, i, :],
                            start=(i == 0),
                            stop=(i == JH - 1),
                        )
                    nc.scalar.activation(
                        g1[:, jj, :],
                        pz1,
                        func=mybir.ActivationFunctionType.Relu,
                        bias=b1t[:, jj : jj + 1],
                        scale=1.0,
                    )

                # layer 2: gs^T = W2^T @ g1 + b2
                gs = epi.tile([PART, JD, K2], f32, tag="gs")
                for j in range(JD):
                    pz2 = ps_mlp.tile([PART, K2], f32, tag="mlp")
                    for i in range(JG):
                        nc.tensor.matmul(
                            pz2,
                            w2_sb[:, i, ts(j, PART)],
                            g1[:, i, :],
                            start=(i == 0),
                            stop=(i == JG - 1),
                        )
                    nc.vector.tensor_scalar_add(gs[:, j, :], pz2, b2t[:, j : j + 1])

                # combine: G^T[:, :K] = S - gs[:K] + gs[K:2K]; G^T[:, K] = S + gs[2K]
                gt = epi.tile([PART, JD, K1], f32, tag="gt")
                ssum = epi.tile([PART, JD], f32, tag="ssum")
                for j in range(JD):
                    nc.vector.reduce_sum(
                        ssum[:, j : j + 1], gs[:, j, 0:K], axis=mybir.AxisListType.X
                    )
                    nc.vector.tensor_sub(gt[:, j, 0:K], gs[:, j, K : 2 * K], gs[:, j, 0:K])
                    nc.vector.tensor_scalar_add(
                        gt[:, j, 0:K], gt[:, j, 0:K], ssum[:, j : j + 1]
                    )
                    nc.vector.tensor_add(
                        gt[:, j, K:K1], gs[:, j, 2 * K : 2 * K + 1], ssum[:, j : j + 1]
                    )

                # back to row-major [K+1, GD] and store
                gout = epi.tile([K1, JD, PART], f32, tag="gout")
                for j in range(JD):
                    pg = ps_mlp.tile([K1, PART], f32, tag="mlp")
                    nc.tensor.transpose(pg, gt[:, j, :], ident)
                    nc.scalar.copy(gout[:, j, :], pg)
                nc.scalar.dma_start(
                    out=out[b], in_=gout.rearrange("k j h -> k (j h)")
                )

            # Defer each batch's epilogue until after the NEXT batch's
            # streaming matmuls are traced: the PE executes in program order,
            # so placing the epilogue's matmuls behind batch b+1's stream
            # lets its cross-engine chain (ACT copy -> PE transpose -> DVE ->
            # PE MLP) fill while the PE keeps grinding stream matmuls,
            # instead of stalling the PE (and starving the DMA pipeline) at
            # every batch boundary.
            prev = None
            for b in range(B_LOC):
                hk = stream_batch(b)
                if prev is not None:
                    epilogue(b - 1, prev)
                prev = hk
            epilogue(B_LOC - 1, prev)

    nc.finalize()
    if split_waits:
        # needed by this toolchain's walrus; CoreSim can't simulate the
        # injected carriers, so sim callers pass split_waits=False
        _split_sync_waits(nc)
    return nc


def kernel(hs, cs_o, W1, b1, W2, b2, n, K):
    global LAST_EXEC_NS, LAST_RESULTS
    from concourse.bass_utils import run_bass_kernel_spmd

    hs = np.ascontiguousarray(np.asarray(hs, dtype=np.float32))
    cs_row = np.ascontiguousarray(np.asarray(cs_o)[0].astype(np.int32))
    W1 = np.ascontiguousarray(np.asarray(W1, dtype=np.float32))
    b1 = np.ascontiguousarray(np.asarray(b1, dtype=np.float32))
    W2 = np.ascontiguousarray(np.asarray(W2, dtype=np.float32))
    b2 = np.ascontiguousarray(np.asarray(b2, dtype=np.float32))
    n = int(n)
    K = int(K)

    B, N, H = hs.shape
    GH = W1.shape[1]
    GD = W2.shape[1]
    assert B % N_CORES == 0
    B_LOC = B // N_CORES

    nc = _build(B_LOC, N, H, GH, GD, K, n, SEG_MODE)

    in_maps = []
    for i in range(N_CORES):
        in_maps.append(
            {
                "hs": hs[i * B_LOC : (i + 1) * B_LOC],
                "cs": cs_row,
                "W1": W1,
                "b1": b1,
                "W2": W2,
                "b2": b2,
            }
        )

    tmpdir = os.environ.get("AGG_TMPDIR") or None
    res = run_bass_kernel_spmd(
        nc, in_maps, core_ids=list(range(N_CORES)), trace=TRACE, tmpdir=tmpdir
    )
    LAST_EXEC_NS = res.exec_time_ns
    LAST_RESULTS = res
    G = np.concatenate([res.results[i]["out"] for i in range(N_CORES)], axis=0)
    G_mask = np.ones((B, K + 1), dtype=np.float32)
    return G, G_mask
